# revision 10
# baseline (speedup 1.0000x reference)
"""Batched 2048-point complex DFT on 8 Trainium2 NeuronCores (v4).

z[b, k] = sum_n x[b, n] W^{nk},  W = exp(-2*pi*i/2048),  x [8192, 2048] fp32.
Data-parallel across 8 cores (1024 rows each).

Factorization n = 16*n1 + n2 (n1 in [0,128), n2 in [0,16)):
  z[b, k] = sum_{n2} W^{n2 k} * u[b, n2, k mod 128]
  u[b, n2, k1] = sum_{n1} x[b, 16 n1 + n2] W128^{n1 k1}      (radix-128 DFT)

Device pipeline per batch-quarter (256 rows, 2 b-tiles):
  1. T-in: PE transposes with stride-16 stationary APs: raw[:, n2::16].T
     -> xt[n1-part, n2, b] (fp32r in, cast to bf16 on PSUM evict).
  2. S1: per n2: 4 bf16 matmuls with the constant radix-128 DFT matrix
     (stationary) -> psum[k1-part, b]; evict (cast bf16) -> y1[k1, n2, b].
  3. Corner turn: SBUF->SBUF DMA per k1-group g: in = y1[8g:8g+8, :, :]
     (partition slice) -> y2_g[p = 16j + n2, b], j = k1 - 8g.
  4. S2 (data-stationary): ps[b, 0:128|128:256] = y2r.T @ [Twr|Twi]
     + y2i.T @ [-Twi|Twr], Tw*[16j+n2, 8k2+j'] = delta_{jj'} W^{n2 k},
     k = 8g + j' + 128 k2.  Output lands [b, k]-oriented.
  5. Scatter-evict psum (cast bf16) into z staging, DMA out bf16;
     host upcasts to fp32.
"""

import contextlib
import os
import sys

sys.path.insert(0, "/opt/trn_rl_repo")
os.environ.setdefault("MYCRO_LOCAL_CACHE", "1")
os.environ.setdefault("JAX_PLATFORMS", "axon,cpu")

import numpy as np
import ml_dtypes

import concourse.bass as bass
import concourse.bacc as bacc
import concourse.mybir as mybir
from concourse import tile
from concourse import bass_utils
from concourse.tile_rust import add_dep_helper

F32 = mybir.dt.float32
F32R = mybir.dt.float32r
BF16 = mybir.dt.bfloat16

N = 2048          # DFT size
B_CORE = 1024     # batch rows per core (8192 / 8)
N_CORES = 8
P = 128

CT_VIA_DRAM = os.environ.get("DFT_CT", "sbuf") == "dram"
DFT_CT_DEPS = os.environ.get("DFT_CT_DEPS", "0") == "1"
DFT_BAR = int(os.environ.get("DFT_BAR", "0"))
DFT_XCAST = os.environ.get("DFT_XCAST", "1") == "1"
DFT_ZQ = os.environ.get("DFT_ZQ", "act")
DFT_PSUM = os.environ.get("DFT_PSUM", "packed")

# ---------------------------------------------------------------- tables ---

def _tables():
    f32 = np.float32
    bf16 = ml_dtypes.bfloat16
    n1 = np.arange(P)[:, None]
    k1 = np.arange(P)[None, :]
    ang1 = -2.0 * np.pi * ((n1 * k1) % P) / P
    bre = np.cos(ang1)
    bim = np.sin(ang1)

    # T2 tables: t2a_g = [Twr | Twi], t2b_g = [-Twi | Twr]
    # partition p = 16 j + n2; column m = half*128 + 8 k2 + j'; delta_{jj'}
    twr = np.zeros((16, P, P), np.float64)
    twi = np.zeros((16, P, P), np.float64)
    for g in range(16):
        for j in range(8):
            n2 = np.arange(16)[:, None]            # partition sub-index
            k2 = np.arange(16)[None, :]
            k = 8 * g + j + 128 * k2               # output frequency
            ang = -2.0 * np.pi * ((n2 * k) % N) / N
            twr[g, 16 * j + n2, 8 * k2 + j] = np.cos(ang)
            twi[g, 16 * j + n2, 8 * k2 + j] = np.sin(ang)
    t2a = np.concatenate([twr, twi], axis=2)       # (16, 128, 256)
    t2b = np.concatenate([-twi, twr], axis=2)
    return {
        "bre": bre.astype(bf16),
        "bim": bim.astype(bf16),
        "bnim": (-bim).astype(bf16),
        "t2a": t2a.reshape(16, P, 2 * P).astype(bf16),
        "t2b": t2b.reshape(16, P, 2 * P).astype(bf16),
        "ident": np.eye(P, dtype=f32),
        "identb": np.eye(P, dtype=bf16),
    }


# ---------------------------------------------------------------- kernel ---

def build_fft_kernel(repeat=1, debug_taps=False):
    nc = bacc.Bacc("TRN2", target_bir_lowering=False, debug=False)

    xr_d = nc.dram_tensor("xr", (B_CORE, N), F32R, kind="ExternalInput")
    xi_d = nc.dram_tensor("xi", (B_CORE, N), F32R, kind="ExternalInput")
    bre_d = nc.dram_tensor("bre", (P, P), BF16, kind="ExternalInput")
    bim_d = nc.dram_tensor("bim", (P, P), BF16, kind="ExternalInput")
    bnim_d = nc.dram_tensor("bnim", (P, P), BF16, kind="ExternalInput")
    t2a_d = nc.dram_tensor("t2a", (16, P, 2 * P), BF16, kind="ExternalInput")
    t2b_d = nc.dram_tensor("t2b", (16, P, 2 * P), BF16, kind="ExternalInput")
    id_d = nc.dram_tensor("ident", (P, P), F32R, kind="ExternalInput")
    idb_d = nc.dram_tensor("identb", (P, P), BF16, kind="ExternalInput")
    zr_d = nc.dram_tensor("zr", (B_CORE, N), BF16, kind="ExternalOutput")
    zi_d = nc.dram_tensor("zi", (B_CORE, N), BF16, kind="ExternalOutput")
    if debug_taps:
        xt_dbg = nc.dram_tensor("xt_dbg", (4, 2, P, 16, 256), BF16, kind="ExternalOutput")
        y1_dbg = nc.dram_tensor("y1_dbg", (4, 2, P, 16, 256), BF16, kind="ExternalOutput")
        y2_dbg = nc.dram_tensor("y2_dbg", (4, 2, 16, P, 256), BF16, kind="ExternalOutput")

    Q = 4             # batch quarters
    BQ = B_CORE // Q  # 256 rows
    NBT = BQ // P     # 2 b-tiles per quarter

    with tile.TileContext(nc) as tc:
        with (
            tc.tile_pool(name="const", bufs=1) as cp,
            tc.tile_pool(name="xraw", bufs=int(os.environ.get("DFT_RAWB", "4"))) as xrp,
            tc.tile_pool(name="xt", bufs=int(os.environ.get("DFT_XTB", "4"))) as xtp,
            tc.tile_pool(name="y1", bufs=int(os.environ.get("DFT_Y1B", "3"))) as y1p,
            tc.tile_pool(name="y2", bufs=int(os.environ.get("DFT_Y2B", "10"))) as y2p,
            tc.tile_pool(name="zstage", bufs=int(os.environ.get("DFT_ZB", "2"))) as zp,
            tc.tile_pool(name="y1d", bufs=2, space="DRAM") as ddp,
            tc.tile_pool(name="tpsum", bufs=int(os.environ.get("DFT_TPB", "2")), space="PSUM") as tpp,
            tc.tile_pool(name="s1psum", bufs=int(os.environ.get("DFT_S1B", "2")), space="PSUM") as s1p,
            tc.tile_pool(
                name="s2psum",
                bufs=(int(os.environ.get("DFT_S2B", "4")) if DFT_PSUM == "packed" else 2),
                space="PSUM",
            ) as s2p,
        ):
            ident = cp.tile([P, P], F32R)
            identb = cp.tile([P, P], BF16)
            nc.sync.dma_start(identb[:], idb_d.ap())
            bre = cp.tile([P, P], BF16)
            bim = cp.tile([P, P], BF16)
            bnim = cp.tile([P, P], BF16)
            t2a = cp.tile([P, 16, 2 * P], BF16)
            t2b = cp.tile([P, 16, 2 * P], BF16)
            nc.sync.dma_start(ident[:], id_d.ap())
            nc.sync.dma_start(bre[:], bre_d.ap())
            nc.sync.dma_start(bim[:], bim_d.ap())
            nc.sync.dma_start(bnim[:], bnim_d.ap())
            nc.sync.dma_start(t2a[:], t2a_d.ap().rearrange("g p m -> p g m"))
            nc.sync.dma_start(t2b[:], t2b_d.ap().rearrange("g p m -> p g m"))

            def ev(i, dst, src):
                if i % 2 == 0:
                    return nc.vector.tensor_copy(dst, src)
                else:
                    return nc.scalar.copy(dst, src)

            rep_ctx = (
                tc.For_i(0, repeat, 1, staggered_reset=os.environ.get("DFT_SRST", "0") == "1") if repeat > 1 else contextlib.nullcontext()
            )
            with rep_ctx:
              for q in range(Q):
                c0 = q * BQ

                # --- load x + transpose-in (stride-16 stationary APs) ---
                xts = []
                for ci, src_d in enumerate((xr_d, xi_d)):
                    xt = xtp.tile([P, 16, BQ], BF16, tag=f"xt{ci}")
                    r_dt = BF16 if DFT_XCAST else F32R
                    r_dma = nc.gpsimd.dma_start if DFT_XCAST else nc.sync.dma_start
                    raw = xrp.tile([P, NBT, N], r_dt, tag="raw")
                    r_dma(
                        raw[:],
                        src_d.ap()[c0 : c0 + BQ, :].rearrange(
                            "(bt p) n -> p bt n", p=P
                        ),
                    )
                    # view cols c = 16*n1 + n2 as [n2, n1]
                    raw_v = raw[:].rearrange("p bt (n1 n2) -> p bt n2 n1", n2=16)
                    raws = [raw_v[:, bt] for bt in range(NBT)]
                    t_dt = BF16 if DFT_XCAST else F32R
                    t_id = identb if DFT_XCAST else ident
                    for n2a in range(0, 16, 2):
                        ps = tpp.tile([P, 4 * P], t_dt, tag="tp")
                        for i, (n2, bt) in enumerate(
                            (n2a + d, b) for d in range(2) for b in range(NBT)
                        ):
                            nc.tensor.matmul(
                                ps[:, i * P : (i + 1) * P],
                                raws[bt][:, n2, :],
                                t_id[:],
                                is_transpose=True,
                                start=(i == 0),
                                stop=(i == 3),
                            )
                        # psum cols: [n2a|bt0, n2a|bt1, n2a+1|bt0, n2a+1|bt1]
                        ev(n2a + ci,
                           xt[:, n2a : n2a + 2, :],
                           ps[:].rearrange("p (n bt b) -> p n (bt b)", n=2, b=P))
                    xts.append(xt)
                    if debug_taps:
                        nc.sync.dma_start(xt_dbg.ap()[q, ci], xt[:])
                xtr, xti = xts

                # --- S1: radix-128 DFT over n1 (constant stationary) ---
                if DFT_BAR:
                    tc.strict_bb_all_engine_barrier()
                y1 = y1p.tile([P, 16, 2, BQ], BF16, tag="y1")
                for n2 in range(16):
                    psRI = s1p.tile([P, 2 * BQ], F32, tag="s1")
                    psR = psRI[:, 0:BQ]
                    psI = psRI[:, BQ : 2 * BQ]
                    nc.tensor.matmul(psR, bre[:], xtr[:, n2, :], start=True, stop=False)
                    nc.tensor.matmul(psR, bnim[:], xti[:, n2, :], start=False, stop=True)
                    nc.tensor.matmul(psI, bre[:], xti[:, n2, :], start=True, stop=False, skip_group_check=True)
                    nc.tensor.matmul(psI, bim[:], xtr[:, n2, :], start=False, stop=True, skip_group_check=True)
                    ev(n2, y1[:, n2, :, :], psRI[:].rearrange("p (c b) -> p c b", c=2))
                if debug_taps:
                    nc.sync.dma_start(y1_dbg.ap()[q, 0], y1r[:])
                    nc.sync.dma_start(y1_dbg.ap()[q, 1], y1i[:])
                if CT_VIA_DRAM:
                    y1rd = ddp.tile([P, 16, BQ], BF16, tag="y1rd")
                    y1id = ddp.tile([P, 16, BQ], BF16, tag="y1id")
                    nc.sync.dma_start(y1rd[:], y1[:, :, 0, :])
                    nc.sync.dma_start(y1id[:], y1[:, :, 1, :])

                # --- corner turn (SBUF->SBUF DMA) + S2 + scatter-evict ---
                zsta = zp.tile([P, NBT, 2, N], BF16, tag="z")
                zv = zsta[:].rearrange(
                    "p bt h (k2 gg j) -> p bt h k2 gg j", k2=16, j=8
                )
                for g in range(16):
                    y2 = y2p.tile([P, 2, BQ], BF16, tag="y2", name=f"y2_{g}")
                    y2r = y2[:, 0, :]
                    y2i = y2[:, 1, :]
                    if CT_VIA_DRAM:
                        nc.sync.dma_start(y2r, y1rd[8 * g : 8 * g + 8, :, :])
                        nc.sync.dma_start(y2i, y1id[8 * g : 8 * g + 8, :, :])
                    else:
                        if os.environ.get("DFT_CTSPLIT") == "1" and g % 2:
                            ct_dma = nc.scalar.dma_start
                        elif os.environ.get("DFT_CT_SWDGE", "1") == "1":
                            ct_dma = nc.gpsimd.dma_start
                        else:
                            ct_dma = nc.sync.dma_start
                        ct_dma(y2[:], y1[8 * g : 8 * g + 8, :, :, :])
                    if debug_taps:
                        nc.sync.dma_start(y2_dbg.ap()[q, 0, g], y2r[:])
                        nc.sync.dma_start(y2_dbg.ap()[q, 1, g], y2i[:])
                    for bt in range(NBT):
                        ps2 = s2p.tile([P, 2 * P], F32, tag="s2")
                        nc.tensor.matmul(
                            ps2[:],
                            y2r[:, bt * P : (bt + 1) * P] if False else y2[:, 0, bt * P : (bt + 1) * P],
                            t2a[:, g, :],
                            start=True,
                            stop=False,
                        )
                        nc.tensor.matmul(
                            ps2[:],
                            y2i[:, bt * P : (bt + 1) * P] if False else y2[:, 1, bt * P : (bt + 1) * P],
                            t2b[:, g, :],
                            start=False,
                            stop=True,
                        )
                        # psum cols m = half*128 + 8*k2 + j  ->  z col 8g+j+128k2
                        if os.environ.get("DFT_SCX") == "1":
                            # timing probe: contiguous evict (WRONG results)
                            ev(g + bt, zsta[:, bt, :, g * P : (g + 1) * P], ps2[:])
                        else:
                            ev(g + bt, zv[:, bt, :, :, g, :],
                               ps2[:].rearrange("p (h k2 j) -> p h k2 j", h=2, j=8))
                zq = nc.scalar.dma_start if DFT_ZQ == "act" else nc.sync.dma_start
                zq(
                    zr_d.ap()[c0 : c0 + BQ, :].rearrange("(bt p) n -> p bt n", p=P),
                    zsta[:, :, 0, :],
                )
                zq(
                    zi_d.ap()[c0 : c0 + BQ, :].rearrange("(bt p) n -> p bt n", p=P),
                    zsta[:, :, 1, :],
                )

    nc.compile()
    return nc


# ---------------------------------------------------------------- driver ---

_NC_CACHE = {}

SCHEME = "fft"


def _get_nc(scheme=None):
    if "fft" not in _NC_CACHE:
        _NC_CACHE["fft"] = build_fft_kernel()
    return _NC_CACHE["fft"]


def make_in_maps(x_real, x_imag, W_real=None, W_imag=None, scheme=None):
    x_real = np.asarray(x_real, dtype=np.float32)
    x_imag = np.asarray(x_imag, dtype=np.float32)
    tabs = _tables()
    in_maps = []
    for c in range(N_CORES):
        sl = slice(c * B_CORE, (c + 1) * B_CORE)
        m = {
            "xr": np.ascontiguousarray(x_real[sl]),
            "xi": np.ascontiguousarray(x_imag[sl]),
        }
        m.update(tabs)
        in_maps.append(m)
    return in_maps


def kernel(x_real, x_imag, W_real=None, W_imag=None):
    nc = _get_nc()
    in_maps = make_in_maps(x_real, x_imag)
    res = bass_utils.run_bass_kernel_spmd(nc, in_maps, core_ids=list(range(N_CORES)))
    zr = np.concatenate(
        [np.asarray(res.results[c]["zr"], dtype=np.float32) for c in range(N_CORES)],
        axis=0,
    )
    zi = np.concatenate(
        [np.asarray(res.results[c]["zi"], dtype=np.float32) for c in range(N_CORES)],
        axis=0,
    )
    return zr, zi
